# revision 33
# baseline (speedup 1.0000x reference)
"""MoE layer (8 experts, top-2, SwiGLU + shared expert) on 8 Trainium2 cores.

Strategy: expert-parallel. Each core holds one expert's weights (full FFN for
expert c) plus a 1/8 slice (over INTER) of the shared expert. Every core:
  1. loads the full token matrix x [4096, 512], transposes it on the PE
     (activations live feature-on-partitions throughout),
  2. computes router scores for all tokens, top-2 renormalized combine
     weights, and extracts the combine-weight column for its own expert,
  3. runs the expert FFN densely over all tokens, scales by the combine
     weight (zero for tokens not routed here), adds its shared-expert slice,
  4. ReduceScatters the [512, 4096] transposed partial across the 8 cores so
     each core ends with a [64, 4096] slice of the final (transposed) output.
Host concatenates the 8 slices and transposes back.
"""

import os

import numpy as np

import concourse.bass as bass
import concourse.bacc as bacc
import concourse.mybir as mybir
from concourse import tile
from concourse.masks import make_identity
from concourse import bass_utils

F32 = mybir.dt.float32
F32R = mybir.dt.float32r   # PE-native reduced fp32: full-rate matmul, ~1e-4 rel err
AF = mybir.ActivationFunctionType
ALU = mybir.AluOpType

# problem shapes (hardcoded per contract)
B, T, H = 2, 2048, 512
E, TOPK, INTER = 8, 2, 1024
N = B * T                      # 4096 tokens
P = 128
NCORES = 8
ISH = INTER // NCORES          # shared-expert INTER shard = 128
HK = H // P                    # 4 k-tiles over H
IT = INTER // P                # 8 i-tiles over INTER
HH = H // P                    # 4 output h-tiles
CHUNK = 512                    # tokens per FFN chunk
NCH = N // CHUNK               # 8 chunks
NBLK = N // P                  # 32 token blocks (router granularity)
NRS = 8                        # reduce-scatter groups over tokens
RSW = N // NRS                 # 512 tokens per RS group
OUTROWS = H // NCORES          # 64 rows of the transposed output per core

# router score accumulation order over the 4 h k-tiles (tweakable: rounding
# order must reproduce the reference's top-2 selection on near-tie tokens)
ROUTER_K_ORDER = [0, 1, 2, 3]

# CoreSim doesn't implement the Silu activation; decompose as x*sigmoid(x)
SIM_COMPAT = False


def build_module():
    nc = bacc.Bacc(
        "TRN2",
        target_bir_lowering=False,
        debug=False,
        enable_asserts=False,
        num_devices=NCORES,
    )

    x_d = nc.dram_tensor("x", [N, H], F32, kind="ExternalInput")
    rw_d = nc.dram_tensor("rw", [H, E], F32, kind="ExternalInput")
    esel_d = nc.dram_tensor("esel", [1, E], F32, kind="ExternalInput")
    wg_d = nc.dram_tensor("wg", [H, INTER], F32, kind="ExternalInput")
    wu_d = nc.dram_tensor("wu", [H, INTER], F32, kind="ExternalInput")
    wd_d = nc.dram_tensor("wd", [INTER, H], F32, kind="ExternalInput")
    sg_d = nc.dram_tensor("sg", [H, ISH], F32, kind="ExternalInput")
    su_d = nc.dram_tensor("su", [H, ISH], F32, kind="ExternalInput")
    sd_d = nc.dram_tensor("sd", [ISH, H], F32, kind="ExternalInput")
    out_d = nc.dram_tensor("out", [OUTROWS, N], F32, kind="ExternalOutput")

    with tile.TileContext(nc) as tc:
        _kernel_body(tc, x_d, rw_d, esel_d, wg_d, wu_d, wd_d, sg_d, su_d, sd_d, out_d)
    nc.compile()
    return nc


def _kernel_body(tc, x_d, rw_d, esel_d, wg_d, wu_d, wd_d, sg_d, su_d, sd_d, out_d):
    nc = tc.nc

    consts = tc.alloc_tile_pool(name="consts", bufs=1)
    wts = tc.alloc_tile_pool(name="wts", bufs=1)
    xT_pool = tc.alloc_tile_pool(name="xT", bufs=1)
    cw_pool = tc.alloc_tile_pool(name="cw", bufs=1)
    dram = tc.alloc_tile_pool(name="dram", bufs=1, space="DRAM")

    identity = consts.tile([P, P], F32)
    make_identity(nc, identity)
    identity_r = consts.tile([P, P], F32R)
    nc.scalar.copy(identity_r, identity)
    esel_sb = consts.tile([P, 1, E], F32)
    nc.sync.dma_start(esel_sb[:, 0, :], esel_d.ap().to_broadcast((P, E)))
    rw_sb = consts.tile([P, HK, E], F32R)
    nc.gpsimd.dma_start(rw_sb, rw_d.ap().rearrange("(k p) e -> p k e", p=P))

    # expert + shared weights, laid out [P, ktile, cols] so each [P, 128] /
    # [P, 512] slice is a ready matmul operand; cast-DMA'd to float32r.
    # DMAs issued after the x-block loads (same SWDGE queue; x is needed first)
    wg_sb = wts.tile([P, HK, INTER], F32R)
    wu_sb = wts.tile([P, HK, INTER], F32R)
    wd_sb = wts.tile([P, IT, H], F32R)
    sg_sb = wts.tile([P, HK, ISH], F32R)
    su_sb = wts.tile([P, HK, ISH], F32R)
    sd_sb = wts.tile([P, H], F32R)

    def load_weights():
        nc.gpsimd.dma_start(wg_sb, wg_d.ap().rearrange("(k p) i -> p k i", p=P))
        nc.gpsimd.dma_start(wu_sb, wu_d.ap().rearrange("(k p) i -> p k i", p=P))
        nc.gpsimd.dma_start(wd_sb, wd_d.ap().rearrange("(k p) h -> p k h", p=P))
        nc.gpsimd.dma_start(sg_sb, sg_d.ap().rearrange("(k p) i -> p k i", p=P))
        nc.gpsimd.dma_start(su_sb, su_d.ap().rearrange("(k p) i -> p k i", p=P))
        nc.gpsimd.dma_start(sd_sb, sd_d.ap())

    xT_sb = xT_pool.tile([P, HK, N], F32R)    # x transposed: [h%128, h//128, tok]
    cw_sb = cw_pool.tile([P, NBLK], F32)      # own-expert combine weight, tok b*128+p
    cwT_sb = cw_pool.tile([NBLK, P], F32)

    # ---- stage 1: transpose x (f32r), batched router, then bulk
    # softmax/top-2 over all 32 blocks at once ----
    sc_all = cw_pool.tile([P, NBLK, E], F32)
    mx_all = cw_pool.tile([P, NBLK, 8], F32)
    with tc.tile_pool(name="s1sb", bufs=4) as s1sb, \
         tc.tile_pool(name="s1ps", bufs=4, space="PSUM") as s1ps:
        # x loads: small first group so transposes start ASAP, 2 MB batches after
        XGROUPS = [(0, 2), (2, 6), (8, 8), (16, 8), (24, 8)]
        for g0, gn in XGROUPS:
            x_sb = s1sb.tile([P, gn, H], F32R, tag="xin", bufs=2,
                             name=f"x_sb_{g0}")
            nc.gpsimd.dma_start(
                x_sb,
                x_d.ap()[g0 * P:(g0 + gn) * P, :].rearrange(
                    "(j p) h -> p j h", p=P),
            )
            for j in range(gn):
                tb = g0 + j
                tp_ps = s1ps.tile([P, HK, P], F32R, tag="tp")
                for hk in range(HK):
                    nc.tensor.transpose(tp_ps[:, hk, :],
                                        x_sb[:, j, hk * P:(hk + 1) * P],
                                        identity_r)
                nc.scalar.copy(xT_sb[:, :, tb * P:(tb + 1) * P], tp_ps)

        load_weights()

        # router: scoresT[e, t] accumulated per 512-token chunk, rw stationary
        for ch in range(NCH):
            scT_ps = s1ps.tile([P, CHUNK], F32, tag="scT", bufs=2)
            for j, hk in enumerate(ROUTER_K_ORDER):
                nc.tensor.matmul(
                    scT_ps[0:E, :],
                    lhsT=rw_sb[:, hk, :],
                    rhs=xT_sb[:, hk, ch * CHUNK:(ch + 1) * CHUNK],
                    start=(j == 0),
                    stop=(j == HK - 1),
                )
            scT_sb = s1sb.tile([E, CHUNK], F32, tag="scT_sb")
            nc.scalar.copy(scT_sb, scT_ps[0:E, :])
            # untranspose scores to [tok, e] blocks
            for b in range(CHUNK // P):
                tb = ch * (CHUNK // P) + b
                tp2_ps = s1ps.tile([P, E], F32, tag="tp2", bufs=2)
                nc.tensor.transpose(tp2_ps, scT_sb[:, b * P:(b + 1) * P],
                                    identity[0:E, 0:E])
                nc.scalar.copy(sc_all[:, tb, :], tp2_ps)

        for tb in range(NBLK):
            nc.vector.max(mx_all[:, tb, :], sc_all[:, tb, :])

        m1 = mx_all[:, :, 0]   # [P, NBLK] strided views
        m2 = mx_all[:, :, 1]
        # top-2 renormalized softmax weights: w1 = 1/(1+e^(m2-m1)), w2 = 1-w1
        d21 = s1sb.tile([P, NBLK], F32, tag="d21")
        nc.vector.tensor_sub(d21, m2, m1)
        e2 = s1sb.tile([P, NBLK], F32, tag="e2")
        nc.scalar.activation(e2, d21, AF.Exp)
        den = s1sb.tile([P, NBLK], F32, tag="den")
        nc.vector.tensor_scalar_add(den, e2, 1.0)
        w1 = s1sb.tile([P, NBLK], F32, tag="w1")
        nc.vector.reciprocal(w1, den)
        w2 = s1sb.tile([P, NBLK], F32, tag="w2")
        nc.vector.tensor_mul(w2, e2, w1)
        # own expert's score & combine weight
        t8 = s1sb.tile([P, NBLK, E], F32, tag="t8")
        nc.vector.tensor_mul(t8, sc_all, esel_sb.to_broadcast((P, NBLK, E)))
        sown = s1sb.tile([P, NBLK], F32, tag="sown")
        nc.vector.reduce_sum(sown, t8, axis=mybir.AxisListType.X)
        eq1 = s1sb.tile([P, NBLK], F32, tag="eq1")
        nc.vector.tensor_tensor(eq1, sown, m1, op=ALU.is_equal)
        eq2 = s1sb.tile([P, NBLK], F32, tag="eq2")
        nc.vector.tensor_tensor(eq2, sown, m2, op=ALU.is_equal)
        nc.vector.tensor_mul(eq1, eq1, w1)
        nc.vector.tensor_mul(eq2, eq2, w2)
        nc.vector.tensor_add(cw_sb, eq1, eq2)

    # ---- stage 2: FFN over token chunks + reduce-scatter ----
    partials = [
        dram.tile([H, RSW], F32, name=f"partial{g}", tag=f"partial{g}")
        for g in range(NRS)
    ]
    rs_outs = [
        dram.tile([OUTROWS, RSW], F32, name=f"rsout{g}", tag=f"rsout{g}")
        for g in range(NRS)
    ]

    with tc.tile_pool(name="s2sb", bufs=2) as s2sb, \
         tc.tile_pool(name="hbuf", bufs=2) as hpool, \
         tc.tile_pool(name="gu_ps", bufs=2, space="PSUM") as gu_ps, \
         tc.tile_pool(name="o_ps", bufs=3, space="PSUM") as o_ps:

        # cw as a row vector in token order: transpose [128, 32] -> [32, 128],
        # bounce through DRAM (linear), read back partition-broadcast per chunk
        cwT_ps = o_ps.tile([P, P], F32, tag="o", name="cwT_ps")
        nc.tensor.transpose(cwT_ps[0:NBLK, :], cw_sb, identity)
        nc.scalar.copy(cwT_sb, cwT_ps[0:NBLK, :])
        cw_dram = dram.tile([NBLK, P], F32, name="cw_dram", tag="cw_dram")
        nc.sync.dma_start(cw_dram, cwT_sb)
        cw_row = cw_dram.rearrange("b p -> (b p)").rearrange("(a t) -> a t", a=1)

        for ch in range(NCH):
            tsl = slice(ch * CHUNK, (ch + 1) * CHUNK)
            # bc[p, t] = cw[t]: DMA partition-broadcast of the cw row slice
            bc_sb = s2sb.tile([P, CHUNK], F32, tag="bc", name=f"bc_{ch}")
            nc.sync.dma_start(bc_sb, cw_row[:, tsl].to_broadcast((P, CHUNK)))

            # shared-expert slice (unscaled)
            gs_ps = gu_ps.tile([P, CHUNK], F32, tag="g", bufs=3)
            us_ps = gu_ps.tile([P, CHUNK], F32, tag="u")
            for hk in range(HK):
                nc.tensor.matmul(gs_ps, lhsT=sg_sb[:, hk, :], rhs=xT_sb[:, hk, tsl],
                                 start=(hk == 0), stop=(hk == HK - 1))
            for hk in range(HK):
                nc.tensor.matmul(us_ps, lhsT=su_sb[:, hk, :], rhs=xT_sb[:, hk, tsl],
                                 start=(hk == 0), stop=(hk == HK - 1))
            ss_sb = s2sb.tile([P, CHUNK], F32, tag="ss")
            if SIM_COMPAT:
                nc.scalar.activation(ss_sb, gs_ps, AF.Sigmoid)
                nc.vector.tensor_mul(ss_sb, ss_sb, gs_ps)
            else:
                nc.scalar.activation(ss_sb, gs_ps, AF.Silu)
            hs_sb = s2sb.tile([P, CHUNK], F32R, tag="hs")
            nc.vector.tensor_mul(hs_sb, ss_sb, us_ps)

            # routed expert, i-tile by i-tile; h is scaled by the combine weight
            hbufs = []
            for it in range(IT):
                g_ps = gu_ps.tile([P, CHUNK], F32, tag="g", name=f"g_{ch}_{it}", bufs=3)
                u_ps = gu_ps.tile([P, CHUNK], F32, tag="u", name=f"u_{ch}_{it}")
                for hk in range(HK):
                    nc.tensor.matmul(g_ps, lhsT=wg_sb[:, hk, it * P:(it + 1) * P],
                                     rhs=xT_sb[:, hk, tsl],
                                     start=(hk == 0), stop=(hk == HK - 1))
                for hk in range(HK):
                    nc.tensor.matmul(u_ps, lhsT=wu_sb[:, hk, it * P:(it + 1) * P],
                                     rhs=xT_sb[:, hk, tsl],
                                     start=(hk == 0), stop=(hk == HK - 1))
                sg_t = s2sb.tile([P, CHUNK], F32, tag="sg_t", name=f"sgt_{ch}_{it}")
                if SIM_COMPAT:
                    nc.scalar.activation(sg_t, g_ps, AF.Sigmoid)
                    nc.vector.tensor_mul(sg_t, sg_t, g_ps)
                else:
                    nc.scalar.activation(sg_t, g_ps, AF.Silu)
                h_t = hpool.tile([P, CHUNK], F32R, name=f"h_{ch}_{it}", tag=f"h{it}")
                nc.vector.tensor_mul(h_t, sg_t, u_ps)
                nc.vector.tensor_mul(h_t, h_t, bc_sb)
                hbufs.append(h_t)

            out_sb = s2sb.tile([P, HH, CHUNK], F32, tag="out")
            for hh in range(HH):
                o_psum = o_ps.tile([P, CHUNK], F32, tag="o", name=f"o_{ch}_{hh}")
                for it in range(IT):
                    nc.tensor.matmul(o_psum, lhsT=wd_sb[:, it, hh * P:(hh + 1) * P],
                                     rhs=hbufs[it], start=(it == 0), stop=False)
                nc.tensor.matmul(o_psum, lhsT=sd_sb[:, hh * P:(hh + 1) * P],
                                 rhs=hs_sb, start=False, stop=True)
                nc.scalar.copy(out_sb[:, hh, :], o_psum)

            g = ch // (NCH // NRS)
            csl = slice((ch % (NCH // NRS)) * CHUNK, (ch % (NCH // NRS) + 1) * CHUNK)
            nc.sync.dma_start(
                partials[g].rearrange("(k p) t -> p k t", p=P)[:, :, csl], out_sb
            )

        for g in range(NRS):
            nc.gpsimd.collective_compute(
                "ReduceScatter",
                ALU.add,
                replica_groups=[list(range(NCORES))],
                ins=[partials[g].opt()],
                outs=[rs_outs[g].opt()],
            )
            nc.sync.dma_start(out_d.ap()[:, g * RSW:(g + 1) * RSW], rs_outs[g])

    for pool in (cw_pool, xT_pool, wts, consts, dram):
        pool.release()


_NC_CACHE = None


def _get_module():
    global _NC_CACHE
    if _NC_CACHE is None:
        _NC_CACHE = build_module()
    return _NC_CACHE


def kernel(x, router_w, Wg, Wu, Wd, Sg, Su, Sd):
    nc = _get_module()
    flat = np.ascontiguousarray(np.asarray(x, dtype=np.float32).reshape(N, H))
    rw = np.ascontiguousarray(np.asarray(router_w, dtype=np.float32))
    Wg = np.asarray(Wg, dtype=np.float32)
    Wu = np.asarray(Wu, dtype=np.float32)
    Wd = np.asarray(Wd, dtype=np.float32)
    Sg = np.asarray(Sg, dtype=np.float32)
    Su = np.asarray(Su, dtype=np.float32)
    Sd = np.asarray(Sd, dtype=np.float32)

    in_maps = []
    for c in range(NCORES):
        esel = np.zeros((1, E), dtype=np.float32)
        esel[0, c] = 1.0
        in_maps.append({
            "x": flat,
            "rw": rw,
            "esel": esel,
            "wg": np.ascontiguousarray(Wg[c]),
            "wu": np.ascontiguousarray(Wu[c]),
            "wd": np.ascontiguousarray(Wd[c]),
            "sg": np.ascontiguousarray(Sg[:, c * ISH:(c + 1) * ISH]),
            "su": np.ascontiguousarray(Su[:, c * ISH:(c + 1) * ISH]),
            "sd": np.ascontiguousarray(Sd[c * ISH:(c + 1) * ISH, :]),
        })

    trace = bool(os.environ.get("MOE_TRACE"))
    res = bass_utils.run_bass_kernel_spmd(
        nc, in_maps, core_ids=list(range(NCORES)), trace=trace
    )
    global LAST_RESULTS
    LAST_RESULTS = res
    outT = np.concatenate([res.results[c]["out"] for c in range(NCORES)], axis=0)
    return np.ascontiguousarray(outT.T).reshape(B, T, H).astype(np.float32)


LAST_RESULTS = None


# revision 34
# speedup vs baseline: 1.0169x; 1.0169x over previous
"""MoE layer (8 experts, top-2, SwiGLU + shared expert) on 8 Trainium2 cores.

Strategy: expert-parallel. Each core holds one expert's weights (full FFN for
expert c) plus a 1/8 slice (over INTER) of the shared expert. Every core:
  1. loads the full token matrix x [4096, 512], transposes it on the PE
     (activations live feature-on-partitions throughout),
  2. computes router scores for all tokens, top-2 renormalized combine
     weights, and extracts the combine-weight column for its own expert,
  3. runs the expert FFN densely over all tokens, scales by the combine
     weight (zero for tokens not routed here), adds its shared-expert slice,
  4. ReduceScatters the [512, 4096] transposed partial across the 8 cores so
     each core ends with a [64, 4096] slice of the final (transposed) output.
Host concatenates the 8 slices and transposes back.
"""

import os

import numpy as np

import concourse.bass as bass
import concourse.bacc as bacc
import concourse.mybir as mybir
from concourse import tile
from concourse.masks import make_identity
from concourse import bass_utils

F32 = mybir.dt.float32
F32R = mybir.dt.float32r   # PE-native reduced fp32: full-rate matmul, ~1e-4 rel err
AF = mybir.ActivationFunctionType
ALU = mybir.AluOpType

# problem shapes (hardcoded per contract)
B, T, H = 2, 2048, 512
E, TOPK, INTER = 8, 2, 1024
N = B * T                      # 4096 tokens
P = 128
NCORES = 8
ISH = INTER // NCORES          # shared-expert INTER shard = 128
HK = H // P                    # 4 k-tiles over H
IT = INTER // P                # 8 i-tiles over INTER
HH = H // P                    # 4 output h-tiles
CHUNK = 512                    # tokens per FFN chunk
NCH = N // CHUNK               # 8 chunks
NBLK = N // P                  # 32 token blocks (router granularity)
NRS = 8                        # reduce-scatter groups over tokens
RSW = N // NRS                 # 512 tokens per RS group
OUTROWS = H // NCORES          # 64 rows of the transposed output per core

# router score accumulation order over the 4 h k-tiles (tweakable: rounding
# order must reproduce the reference's top-2 selection on near-tie tokens)
ROUTER_K_ORDER = [0, 1, 2, 3]

# CoreSim doesn't implement the Silu activation; decompose as x*sigmoid(x)
SIM_COMPAT = False


def build_module():
    nc = bacc.Bacc(
        "TRN2",
        target_bir_lowering=False,
        debug=False,
        enable_asserts=False,
        num_devices=NCORES,
    )

    x_d = nc.dram_tensor("x", [N, H], F32, kind="ExternalInput")
    rw_d = nc.dram_tensor("rw", [H, E], F32, kind="ExternalInput")
    esel_d = nc.dram_tensor("esel", [1, E], F32, kind="ExternalInput")
    wg_d = nc.dram_tensor("wg", [H, INTER], F32, kind="ExternalInput")
    wu_d = nc.dram_tensor("wu", [H, INTER], F32, kind="ExternalInput")
    wd_d = nc.dram_tensor("wd", [INTER, H], F32, kind="ExternalInput")
    sg_d = nc.dram_tensor("sg", [H, ISH], F32, kind="ExternalInput")
    su_d = nc.dram_tensor("su", [H, ISH], F32, kind="ExternalInput")
    sd_d = nc.dram_tensor("sd", [ISH, H], F32, kind="ExternalInput")
    out_d = nc.dram_tensor("out", [OUTROWS, N], F32, kind="ExternalOutput")

    with tile.TileContext(nc) as tc:
        _kernel_body(tc, x_d, rw_d, esel_d, wg_d, wu_d, wd_d, sg_d, su_d, sd_d, out_d)
    nc.compile()
    return nc


def _kernel_body(tc, x_d, rw_d, esel_d, wg_d, wu_d, wd_d, sg_d, su_d, sd_d, out_d):
    nc = tc.nc

    consts = tc.alloc_tile_pool(name="consts", bufs=1)
    wts = tc.alloc_tile_pool(name="wts", bufs=1)
    xT_pool = tc.alloc_tile_pool(name="xT", bufs=1)
    cw_pool = tc.alloc_tile_pool(name="cw", bufs=1)
    dram = tc.alloc_tile_pool(name="dram", bufs=1, space="DRAM")

    identity = consts.tile([P, P], F32)
    make_identity(nc, identity)
    identity_r = consts.tile([P, P], F32R)
    nc.scalar.copy(identity_r, identity)
    esel_sb = consts.tile([P, 1, E], F32)
    nc.sync.dma_start(esel_sb[:, 0, :], esel_d.ap().to_broadcast((P, E)))
    rw_sb = consts.tile([P, HK, E], F32R)
    nc.gpsimd.dma_start(rw_sb, rw_d.ap().rearrange("(k p) e -> p k e", p=P))

    # expert + shared weights, laid out [P, ktile, cols] so each [P, 128] /
    # [P, 512] slice is a ready matmul operand; cast-DMA'd to float32r.
    # DMAs issued after the x-block loads (same SWDGE queue; x is needed first)
    wg_sb = wts.tile([P, HK, INTER], F32R)
    wu_sb = wts.tile([P, HK, INTER], F32R)
    wd_sb = wts.tile([P, IT, H], F32R)
    sg_sb = wts.tile([P, HK, ISH], F32R)
    su_sb = wts.tile([P, HK, ISH], F32R)
    sd_sb = wts.tile([P, H], F32R)

    def load_weights():
        nc.gpsimd.dma_start(wg_sb, wg_d.ap().rearrange("(k p) i -> p k i", p=P))
        nc.gpsimd.dma_start(wu_sb, wu_d.ap().rearrange("(k p) i -> p k i", p=P))
        nc.gpsimd.dma_start(wd_sb, wd_d.ap().rearrange("(k p) h -> p k h", p=P))
        nc.gpsimd.dma_start(sg_sb, sg_d.ap().rearrange("(k p) i -> p k i", p=P))
        nc.gpsimd.dma_start(su_sb, su_d.ap().rearrange("(k p) i -> p k i", p=P))
        nc.gpsimd.dma_start(sd_sb, sd_d.ap())

    xT_sb = xT_pool.tile([P, HK, N], F32R)    # x transposed: [h%128, h//128, tok]
    cw_sb = cw_pool.tile([P, NBLK], F32)      # own-expert combine weight, tok b*128+p
    cwT_sb = cw_pool.tile([NBLK, P], F32)

    # ---- stage 1: transpose x (f32r), batched router, then bulk
    # softmax/top-2 over all 32 blocks at once ----
    sc_all = cw_pool.tile([P, NBLK, E], F32)
    mx_all = cw_pool.tile([P, NBLK, 8], F32)
    with tc.tile_pool(name="s1sb", bufs=4) as s1sb, \
         tc.tile_pool(name="s1ps", bufs=4, space="PSUM") as s1ps:
        # x loads: small first group so transposes start ASAP, 1 MB batches after
        XGROUPS = [(0, 2), (2, 4), (6, 4), (10, 4), (14, 4), (18, 4),
                   (22, 4), (26, 4), (30, 2)]
        for g0, gn in XGROUPS:
            x_sb = s1sb.tile([P, gn, H], F32R, tag="xin", bufs=2,
                             name=f"x_sb_{g0}")
            nc.gpsimd.dma_start(
                x_sb,
                x_d.ap()[g0 * P:(g0 + gn) * P, :].rearrange(
                    "(j p) h -> p j h", p=P),
            )
            for j in range(gn):
                tb = g0 + j
                tp_ps = s1ps.tile([P, HK, P], F32R, tag="tp")
                for hk in range(HK):
                    nc.tensor.transpose(tp_ps[:, hk, :],
                                        x_sb[:, j, hk * P:(hk + 1) * P],
                                        identity_r)
                nc.scalar.copy(xT_sb[:, :, tb * P:(tb + 1) * P], tp_ps)

        load_weights()

        # router: scoresT[e, t] accumulated per 512-token chunk, rw stationary
        for ch in range(NCH):
            scT_ps = s1ps.tile([P, CHUNK], F32, tag="scT", bufs=2)
            for j, hk in enumerate(ROUTER_K_ORDER):
                nc.tensor.matmul(
                    scT_ps[0:E, :],
                    lhsT=rw_sb[:, hk, :],
                    rhs=xT_sb[:, hk, ch * CHUNK:(ch + 1) * CHUNK],
                    start=(j == 0),
                    stop=(j == HK - 1),
                )
            scT_sb = s1sb.tile([E, CHUNK], F32, tag="scT_sb")
            nc.scalar.copy(scT_sb, scT_ps[0:E, :])
            # untranspose scores to [tok, e] blocks
            for b in range(CHUNK // P):
                tb = ch * (CHUNK // P) + b
                tp2_ps = s1ps.tile([P, E], F32, tag="tp2", bufs=2)
                nc.tensor.transpose(tp2_ps, scT_sb[:, b * P:(b + 1) * P],
                                    identity[0:E, 0:E])
                nc.scalar.copy(sc_all[:, tb, :], tp2_ps)

        for tb in range(NBLK):
            nc.vector.max(mx_all[:, tb, :], sc_all[:, tb, :])

        m1 = mx_all[:, :, 0]   # [P, NBLK] strided views
        m2 = mx_all[:, :, 1]
        # top-2 renormalized softmax weights: w1 = 1/(1+e^(m2-m1)), w2 = 1-w1
        d21 = s1sb.tile([P, NBLK], F32, tag="d21")
        nc.vector.tensor_sub(d21, m2, m1)
        e2 = s1sb.tile([P, NBLK], F32, tag="e2")
        nc.scalar.activation(e2, d21, AF.Exp)
        den = s1sb.tile([P, NBLK], F32, tag="den")
        nc.vector.tensor_scalar_add(den, e2, 1.0)
        w1 = s1sb.tile([P, NBLK], F32, tag="w1")
        nc.vector.reciprocal(w1, den)
        w2 = s1sb.tile([P, NBLK], F32, tag="w2")
        nc.vector.tensor_mul(w2, e2, w1)
        # own expert's score & combine weight
        t8 = s1sb.tile([P, NBLK, E], F32, tag="t8")
        nc.vector.tensor_mul(t8, sc_all, esel_sb.to_broadcast((P, NBLK, E)))
        sown = s1sb.tile([P, NBLK], F32, tag="sown")
        nc.vector.reduce_sum(sown, t8, axis=mybir.AxisListType.X)
        eq1 = s1sb.tile([P, NBLK], F32, tag="eq1")
        nc.vector.tensor_tensor(eq1, sown, m1, op=ALU.is_equal)
        eq2 = s1sb.tile([P, NBLK], F32, tag="eq2")
        nc.vector.tensor_tensor(eq2, sown, m2, op=ALU.is_equal)
        nc.vector.tensor_mul(eq1, eq1, w1)
        nc.vector.tensor_mul(eq2, eq2, w2)
        nc.vector.tensor_add(cw_sb, eq1, eq2)

    # ---- stage 2: FFN over token chunks + reduce-scatter ----
    partials = [
        dram.tile([H, RSW], F32, name=f"partial{g}", tag=f"partial{g}")
        for g in range(NRS)
    ]
    rs_outs = [
        dram.tile([OUTROWS, RSW], F32, name=f"rsout{g}", tag=f"rsout{g}")
        for g in range(NRS)
    ]

    with tc.tile_pool(name="s2sb", bufs=2) as s2sb, \
         tc.tile_pool(name="hbuf", bufs=2) as hpool, \
         tc.tile_pool(name="gu_ps", bufs=2, space="PSUM") as gu_ps, \
         tc.tile_pool(name="o_ps", bufs=3, space="PSUM") as o_ps:

        # cw as a row vector in token order: transpose [128, 32] -> [32, 128],
        # bounce through DRAM (linear), read back partition-broadcast per chunk
        cwT_ps = o_ps.tile([P, P], F32, tag="o", name="cwT_ps")
        nc.tensor.transpose(cwT_ps[0:NBLK, :], cw_sb, identity)
        nc.scalar.copy(cwT_sb, cwT_ps[0:NBLK, :])
        cw_dram = dram.tile([NBLK, P], F32, name="cw_dram", tag="cw_dram")
        nc.sync.dma_start(cw_dram, cwT_sb)
        cw_row = cw_dram.rearrange("b p -> (b p)").rearrange("(a t) -> a t", a=1)

        for ch in range(NCH):
            tsl = slice(ch * CHUNK, (ch + 1) * CHUNK)
            # bc[p, t] = cw[t]: DMA partition-broadcast of the cw row slice
            bc_sb = s2sb.tile([P, CHUNK], F32, tag="bc", name=f"bc_{ch}")
            nc.sync.dma_start(bc_sb, cw_row[:, tsl].to_broadcast((P, CHUNK)))

            # shared-expert slice (unscaled)
            gs_ps = gu_ps.tile([P, CHUNK], F32, tag="g", bufs=3)
            us_ps = gu_ps.tile([P, CHUNK], F32, tag="u")
            for hk in range(HK):
                nc.tensor.matmul(gs_ps, lhsT=sg_sb[:, hk, :], rhs=xT_sb[:, hk, tsl],
                                 start=(hk == 0), stop=(hk == HK - 1))
            for hk in range(HK):
                nc.tensor.matmul(us_ps, lhsT=su_sb[:, hk, :], rhs=xT_sb[:, hk, tsl],
                                 start=(hk == 0), stop=(hk == HK - 1))
            ss_sb = s2sb.tile([P, CHUNK], F32, tag="ss")
            if SIM_COMPAT:
                nc.scalar.activation(ss_sb, gs_ps, AF.Sigmoid)
                nc.vector.tensor_mul(ss_sb, ss_sb, gs_ps)
            else:
                nc.scalar.activation(ss_sb, gs_ps, AF.Silu)
            hs_sb = s2sb.tile([P, CHUNK], F32R, tag="hs")
            nc.vector.tensor_mul(hs_sb, ss_sb, us_ps)

            # routed expert, i-tile by i-tile; h is scaled by the combine weight
            hbufs = []
            for it in range(IT):
                g_ps = gu_ps.tile([P, CHUNK], F32, tag="g", name=f"g_{ch}_{it}", bufs=3)
                u_ps = gu_ps.tile([P, CHUNK], F32, tag="u", name=f"u_{ch}_{it}")
                for hk in range(HK):
                    nc.tensor.matmul(g_ps, lhsT=wg_sb[:, hk, it * P:(it + 1) * P],
                                     rhs=xT_sb[:, hk, tsl],
                                     start=(hk == 0), stop=(hk == HK - 1))
                for hk in range(HK):
                    nc.tensor.matmul(u_ps, lhsT=wu_sb[:, hk, it * P:(it + 1) * P],
                                     rhs=xT_sb[:, hk, tsl],
                                     start=(hk == 0), stop=(hk == HK - 1))
                sg_t = s2sb.tile([P, CHUNK], F32, tag="sg_t", name=f"sgt_{ch}_{it}")
                if SIM_COMPAT:
                    nc.scalar.activation(sg_t, g_ps, AF.Sigmoid)
                    nc.vector.tensor_mul(sg_t, sg_t, g_ps)
                else:
                    nc.scalar.activation(sg_t, g_ps, AF.Silu)
                h_t = hpool.tile([P, CHUNK], F32R, name=f"h_{ch}_{it}", tag=f"h{it}")
                nc.vector.tensor_mul(h_t, sg_t, u_ps)
                nc.vector.tensor_mul(h_t, h_t, bc_sb)
                hbufs.append(h_t)

            out_sb = s2sb.tile([P, HH, CHUNK], F32, tag="out")
            for hh in range(HH):
                o_psum = o_ps.tile([P, CHUNK], F32, tag="o", name=f"o_{ch}_{hh}")
                for it in range(IT):
                    nc.tensor.matmul(o_psum, lhsT=wd_sb[:, it, hh * P:(hh + 1) * P],
                                     rhs=hbufs[it], start=(it == 0), stop=False)
                nc.tensor.matmul(o_psum, lhsT=sd_sb[:, hh * P:(hh + 1) * P],
                                 rhs=hs_sb, start=False, stop=True)
                nc.scalar.copy(out_sb[:, hh, :], o_psum)

            g = ch // (NCH // NRS)
            csl = slice((ch % (NCH // NRS)) * CHUNK, (ch % (NCH // NRS) + 1) * CHUNK)
            nc.sync.dma_start(
                partials[g].rearrange("(k p) t -> p k t", p=P)[:, :, csl], out_sb
            )

        for g in range(NRS):
            nc.gpsimd.collective_compute(
                "ReduceScatter",
                ALU.add,
                replica_groups=[list(range(NCORES))],
                ins=[partials[g].opt()],
                outs=[rs_outs[g].opt()],
            )
            nc.sync.dma_start(out_d.ap()[:, g * RSW:(g + 1) * RSW], rs_outs[g])

    for pool in (cw_pool, xT_pool, wts, consts, dram):
        pool.release()


_NC_CACHE = None


def _get_module():
    global _NC_CACHE
    if _NC_CACHE is None:
        _NC_CACHE = build_module()
    return _NC_CACHE


def kernel(x, router_w, Wg, Wu, Wd, Sg, Su, Sd):
    nc = _get_module()
    flat = np.ascontiguousarray(np.asarray(x, dtype=np.float32).reshape(N, H))
    rw = np.ascontiguousarray(np.asarray(router_w, dtype=np.float32))
    Wg = np.asarray(Wg, dtype=np.float32)
    Wu = np.asarray(Wu, dtype=np.float32)
    Wd = np.asarray(Wd, dtype=np.float32)
    Sg = np.asarray(Sg, dtype=np.float32)
    Su = np.asarray(Su, dtype=np.float32)
    Sd = np.asarray(Sd, dtype=np.float32)

    in_maps = []
    for c in range(NCORES):
        esel = np.zeros((1, E), dtype=np.float32)
        esel[0, c] = 1.0
        in_maps.append({
            "x": flat,
            "rw": rw,
            "esel": esel,
            "wg": np.ascontiguousarray(Wg[c]),
            "wu": np.ascontiguousarray(Wu[c]),
            "wd": np.ascontiguousarray(Wd[c]),
            "sg": np.ascontiguousarray(Sg[:, c * ISH:(c + 1) * ISH]),
            "su": np.ascontiguousarray(Su[:, c * ISH:(c + 1) * ISH]),
            "sd": np.ascontiguousarray(Sd[c * ISH:(c + 1) * ISH, :]),
        })

    trace = bool(os.environ.get("MOE_TRACE"))
    res = bass_utils.run_bass_kernel_spmd(
        nc, in_maps, core_ids=list(range(NCORES)), trace=trace
    )
    global LAST_RESULTS
    LAST_RESULTS = res
    outT = np.concatenate([res.results[c]["out"] for c in range(NCORES)], axis=0)
    return np.ascontiguousarray(outT.T).reshape(B, T, H).astype(np.float32)


LAST_RESULTS = None
